# revision 18
# baseline (speedup 1.0000x reference)
"""Pfaffian of skew-symmetric auxiliary-fermion matrix on Trainium2.

Blocked right-looking pair elimination (Parlett-Reid style):
  - host: pivoted factorization of each 128x128 diagonal block (O(n*nb^2))
  - device (8 NeuronCores): trailing Schur update S += Cp^T (Bpinv Cp),
    column-sharded across cores (the O(n^3) bulk).
Pfaffian = (-1)^nswaps * prod(pivots), accumulated sequentially in f32 to
match the reference's f32 product semantics (overflow behavior included).
"""

import numpy as np

NPARTICLE = 1024
NFREE = 128
NA = NPARTICLE + NFREE  # 1152
NB = 128                # panel size
NT = NA - NB            # 1024, max (padded) trailing size
NCORES = 8
CSLICE = NT // NCORES   # 128 columns per core
MCHUNKS = NT // 128     # 8 row chunks of the trailing slab


def _build_A(x, F, U):
    idx = np.flatnonzero(x > 0)[:NPARTICLE]
    a = idx[:, None]
    b = idx[None, :]
    lo = np.minimum(a, b)
    hi = np.maximum(a, b)
    off = (hi.astype(np.int64) * (hi.astype(np.int64) - 1)) // 2 + lo
    vals = F[off]
    blk = np.where(a > b, vals, -vals).astype(np.float32)
    np.fill_diagonal(blk, 0.0)
    sd = U[idx].astype(np.float32)
    A = np.zeros((NA, NA), np.float32)
    A[:NPARTICLE, :NPARTICLE] = blk
    A[:NPARTICLE, NPARTICLE:] = sd
    A[NPARTICLE:, :NPARTICLE] = -sd.T
    return A


def _factor_block(B0):
    """Pair elimination with partial pivoting on a 2m x 2m skew f32 block.
    Returns (pivots, nswap, perm, Binv) with Binv = P @ B^{-1} (row-permuted
    inverse); inverse of the permuted block Bp = P B P^T is Binv[:, perm]."""
    m = B0.shape[0]
    dt = np.float32
    perm = np.arange(m)
    pivots = np.empty(m // 2, dt)
    nswap = 0
    Aug = np.concatenate([B0.astype(dt), np.eye(m, dtype=dt)], axis=1)
    for j in range(0, m, 2):
        r = j + 1 + int(np.argmax(np.abs(Aug[j + 1:m, j])))
        if r != j + 1:
            Aug[[j + 1, r], :] = Aug[[r, j + 1], :]
            Aug[:, [j + 1, r]] = Aug[:, [r, j + 1]]
            perm[[j + 1, r]] = perm[[r, j + 1]]
            nswap += 1
        p = Aug[j, j + 1]
        pivots[j // 2] = p
        u = Aug[j, :].copy()
        v = Aug[j + 1, :].copy()
        c0 = Aug[j + 2:m, j].copy()
        c1 = Aug[j + 2:m, j + 1].copy()
        Aug[j + 2:m, :] += (np.outer(c0, v) - np.outer(c1, u)) / p
    for j in range(m - 2, -1, -2):
        u = Aug[j, :].copy()
        v = Aug[j + 1, :].copy()
        p = Aug[j, j + 1]
        c0 = Aug[:j, j].copy()
        c1 = Aug[:j, j + 1].copy()
        Aug[:j, :] += (np.outer(c0, v) - np.outer(c1, u)) / p
    M = Aug[:, m:]
    Binv = np.empty_like(M)
    for j in range(0, m, 2):
        p = Aug[j, j + 1]
        Binv[j, :] = -M[j + 1, :] / p
        Binv[j + 1, :] = M[j, :] / p
    return pivots, nswap, perm, Binv


_NC = None


def _get_nc():
    """Build (once) the Bass program for one trailing update on one core:
      X = Bpinv @ Cp_slice            (128x128 @ 128x128)
      Sout = St + Cp^T @ X            (1024x128, as 8 chunks of 128x128)
    """
    global _NC
    if _NC is not None:
        return _NC
    import concourse.bass as bass
    from concourse import tile
    from concourse import mybir

    f32 = mybir.dt.float32
    nc = bass.Bass()
    # inp = [ Bpinv^T (128) | Cp_slice (128) | Cp (1024) ] along free dim
    inp = nc.declare_dram_parameter("inp", (NB, 2 * NB + NT), f32, isOutput=False)
    st = nc.declare_dram_parameter("st", (128, NT), f32, isOutput=False)
    sout = nc.declare_dram_parameter("sout", (128, NT), f32, isOutput=True)

    # HW limit: every instruction (incl. DMA) supports at most ONE semaphore
    # wait; Tile never elides same-engine RAW deps but merges same-sem waits.
    # So: 3 total DMAs (each lands on its own ring, no ring-order wait), both
    # TT inputs staged through DVE so each TT carries one merged DVE wait, and
    # a single output DMA whose 8 TT deps merge into one DVE wait.
    with tile.TileContext(nc) as tc:
        with (
            tc.tile_pool(name="pers", bufs=1) as pers,
            tc.tile_pool(name="work", bufs=24) as work,
            tc.tile_pool(name="ps", bufs=4, space=bass.MemorySpace.PSUM) as ps,
        ):
            inp_t = pers.tile([NB, 2 * NB + NT], f32)
            nc.gpsimd.dma_start(inp_t[:], inp[:])
            st_big = pers.tile([128, NT], f32)
            nc.gpsimd.dma_start(st_big[:], st[:])
            out_big = pers.tile([128, NT], f32)

            bp_t = inp_t[:, 0:NB]
            cps_t = inp_t[:, NB:2 * NB]
            x_ps = ps.tile([NB, CSLICE], f32)
            nc.tensor.matmul(x_ps[:], bp_t, cps_t)
            x_sb = pers.tile([NB, CSLICE], f32)
            nc.vector.tensor_copy(x_sb[:], x_ps[:])

            for mi in range(MCHUNKS):
                sl = slice(mi * 128, (mi + 1) * 128)
                o_ps = ps.tile([128, CSLICE], f32)
                lo = 2 * NB + mi * 128
                nc.tensor.matmul(o_ps[:], inp_t[:, lo:lo + 128], x_sb[:])
                st_sb = work.tile([128, CSLICE], f32)
                nc.vector.tensor_copy(st_sb[:], st_big[:, sl])
                o_sb = work.tile([128, CSLICE], f32)
                nc.vector.tensor_copy(o_sb[:], o_ps[:])
                nc.vector.tensor_tensor(out_big[:, sl], o_sb[:], st_sb[:],
                                        op=mybir.AluOpType.add)
            nc.gpsimd.dma_start(sout[:], out_big[:])

    # codegen rejects >1 sem wait on ANY instruction (incl. the final Drain);
    # split extra waits into single-wait NoOps on the same queue just before
    for blk in nc.m.functions[0].blocks:
        newlist = []
        for ins in blk.instructions:
            si = getattr(ins, "sync_info", None)
            w = list(si.on_wait) if (si and si.on_wait) else []
            if len(w) > 1:
                for i, sw in enumerate(w[:-1]):
                    nop = mybir.InstNoOp(
                        name=f"{ins.name}-wsplit{i}",
                        sync_info=mybir.SyncInfo(on_wait=[sw], on_update=[]),
                        bass_nofuse=True,
                        engine=ins.engine,
                    )
                    nc.register_instruction(nop, overwrite=True)
                    newlist.append(nop)
                ins.sync_info = mybir.SyncInfo(
                    on_wait=[w[-1]], on_update=si.on_update)
            newlist.append(ins)
        blk.instructions[:] = newlist
    _NC = nc
    return nc


LAST_HW_NS = None
LAST_DEVICE_WALL_NS = 0


def kernel(x, F, U):
    global LAST_HW_NS, LAST_DEVICE_WALL_NS
    import time
    from concourse.bass_utils import run_bass_kernel_spmd

    LAST_HW_NS = None
    LAST_DEVICE_WALL_NS = 0
    A = _build_A(np.asarray(x), np.asarray(F), np.asarray(U))
    nc = _get_nc()
    core_ids = list(range(NCORES))
    half = np.float32(0.5)

    piv_all = []
    nswap = 0
    for k0 in range(0, NA, NB):
        k1 = k0 + NB
        Bblk = A[k0:k1, k0:k1]
        Bblk = ((Bblk - Bblk.T) * half).astype(np.float32)
        np.fill_diagonal(Bblk, 0.0)
        piv, ns, perm, Binv = _factor_block(Bblk)
        piv_all.append(piv)
        nswap += ns
        if k1 >= NA:
            break
        nt = NA - k1
        Cp = A[k0:k1, k1:][perm]
        Bpinv = Binv[:, perm]
        Bpinv = ((Bpinv - Bpinv.T) * half).astype(np.float32)

        cp_pad = np.zeros((NB, NT), np.float32)
        cp_pad[:, :nt] = Cp
        s_pad = np.zeros((NT, NT), np.float32)
        s_pad[:nt, :nt] = A[k1:, k1:]

        bp_T = np.ascontiguousarray(Bpinv.T)
        in_maps = []
        for c in range(NCORES):
            sl = slice(c * CSLICE, (c + 1) * CSLICE)
            in_maps.append({
                "inp": np.concatenate(
                    [bp_T, cp_pad[:, sl], cp_pad], axis=1).astype(np.float32),
                "st": np.ascontiguousarray(
                    s_pad[:, sl].reshape(MCHUNKS, 128, CSLICE)
                    .transpose(1, 0, 2).reshape(128, NT)),
            })
        t0 = time.perf_counter_ns()
        res = run_bass_kernel_spmd(nc, in_maps, core_ids)
        LAST_DEVICE_WALL_NS += time.perf_counter_ns() - t0
        if res.exec_time_ns is not None:
            LAST_HW_NS = (LAST_HW_NS or 0) + res.exec_time_ns
        for c in range(NCORES):
            s_pad[:, c * CSLICE:(c + 1) * CSLICE] = (
                np.asarray(res.results[c]["sout"])
                .reshape(128, MCHUNKS, CSLICE)
                .transpose(1, 0, 2).reshape(NT, CSLICE))
        A[k1:, k1:] = s_pad[:nt, :nt]

    pivs = np.concatenate(piv_all).astype(np.float32)
    pf = np.float32(-1.0) if (nswap % 2) else np.float32(1.0)
    for p in pivs:
        pf = np.float32(pf * p)
    return np.asarray(pf, dtype=np.float32)
